# revision 1
# baseline (speedup 1.0000x reference)
"""Trainium2 Bass kernel for LinearAttention (B=8, S=4096, D=512, H=8, DH=64).

Sharding: data-parallel over batch — core b processes batch element b end-to-end.

Per-core pipeline (matmuls in f32r = full-rate reduced-precision fp32):
  pass A (per 512-wide s-chunk):
    x tile -> PE-transpose (f32) -> xT (rounded to f32r)
    qT = Wq^T x^T (psum), phi -> QfT [inner, s]  (bias via ACT per-partition bias)
    k  = x Wk + bias (ones-row matmul), phi -> Kf [s, inner]
    v  = x Wv, +bias fused in the psum->sbuf copy -> v' (quad layout + ones col)
    KV' accumulated over all of S per (quad g, chunk cc) psum tile:
        KVq[g,cc] += Kf[:,chunk]^T @ v'[:,quad]   (ones col produces Ksum)
  pass B (per 256-wide s-slice):
    den = Qf . Ksum (block-diag rhs), Z = 1/(den+eps)  [s,8] natural
    Z^T via PE transpose (f32), Zrep via selector matmul (E8)
    O^T = KV^T @ QfT per head (pairs packed diagonally via tile_position)
    OT = O^T * Zrep (DVE) -> out = OT^T Wo + bo
"""

import os
import sys

import numpy as np

for _p in ("/opt/trn_rl_repo",):
    if os.path.isdir(_p) and _p not in sys.path:
        sys.path.insert(0, _p)

from contextlib import ExitStack

import concourse.bass as bass
import concourse.mybir as mybir
import concourse.tile as tile
from concourse.bass_utils import run_bass_kernel_spmd
from concourse.masks import make_identity
from concourse import library_config

B, S, D = 8, 4096, 512
H, DH = 8, 64
INNER = H * DH  # 512
EPS = 1e-6

F32 = mybir.dt.float32
F32R = mybir.dt.float32r
AF = mybir.ActivationFunctionType
ALU = mybir.AluOpType

# matmul input dtype: "f32r" (full-speed, reduced precision) or "f32" (4x slower)
MM_DTYPE = os.environ.get("LINATTN_MM_DTYPE", "f32r")
DT_MM = F32R if MM_DTYPE == "f32r" else F32


def _linattn_body(ctx: ExitStack, tc: "tile.TileContext", io: dict, s_total: int, reps: int = 1):
    nc = tc.nc
    NT = s_total // 128  # s-tiles
    NCH = s_total // 512  # pass-A chunks
    NSL = s_total // 256  # pass-B slices

    x_d = io["x"]
    out_d = io["out"]

    singles = ctx.enter_context(tc.tile_pool(name="singles", bufs=1))

    # ---- weights: [128, 4, 512] in DT_MM; chunk c holds rows c*128..(c+1)*128 ----
    # DMA lands f32; a DVE copy rounds into the matmul dtype.
    w_sb = {}
    with tc.tile_pool(name="wraw_pool", bufs=2) as wraw_pool:
        for name in ("Wq", "Wk", "Wv", "Wo"):
            raw = wraw_pool.tile([128, 4, INNER], F32, tag="wraw")
            w3 = io[name].rearrange("(c p) n -> c p n", p=128)
            t = singles.tile([128, 4, INNER], DT_MM, name=f"{name}_sb", tag=f"{name}_sb")
            for c in range(4):
                nc.sync.dma_start(out=raw[:, c, :], in_=w3[c])
                nc.vector.tensor_copy(out=t[:, c, :], in_=raw[:, c, :])
            w_sb[name] = t
        bk_raw = wraw_pool.tile([1, INNER], F32, tag="bk_raw")
        nc.gpsimd.dma_start(out=bk_raw[:], in_=io["bk"].rearrange("(a n) -> a n", a=1))
        bk_row = singles.tile([1, INNER], DT_MM, name="bk_row", tag="bk_row")
        nc.vector.tensor_copy(out=bk_row[:], in_=bk_raw[:])

    # ---- biases ----
    bq_sb = singles.tile([128, 4], F32, name="bq_sb", tag="bq_sb")
    nc.gpsimd.dma_start(out=bq_sb[:], in_=io["bq"].rearrange("(c p) -> p c", p=128))
    # bv/bo replicated across partitions via partition-step-0 DMA
    bv_rep = singles.tile([128, INNER], F32, name="bv_rep", tag="bv_rep")
    bv_ap = io["bv"]
    nc.gpsimd.dma_start(
        out=bv_rep[:],
        in_=bass.AP(tensor=bv_ap.tensor, offset=bv_ap.offset, ap=[[0, 128]] + list(bv_ap.ap)),
    )
    bo_rep = singles.tile([128, D], F32, name="bo_rep", tag="bo_rep")
    bo_ap = io["bo"]
    nc.gpsimd.dma_start(
        out=bo_rep[:],
        in_=bass.AP(tensor=bo_ap.tensor, offset=bo_ap.offset, ap=[[0, 128]] + list(bo_ap.ap)),
    )

    # ---- constants ----
    ident = singles.tile([128, 128], F32, name="ident", tag="ident")
    make_identity(nc, ident[:])
    # GPSIMD memset/affine_select cannot write f32r: stage in F32, DVE-copy over.
    ones_col = singles.tile([1, 128], DT_MM, name="ones_col", tag="ones_col")
    ones_vcol = singles.tile([128, 2, 2], F32, name="ones_vcol", tag="ones_vcol")
    e8 = singles.tile([8, 4, 128], DT_MM, name="e8", tag="e8")
    with tc.tile_pool(name="const_stage", bufs=4) as cst:
        st1 = cst.tile([1, 128], F32, tag="st1")
        nc.gpsimd.memset(st1[:], 1.0)
        nc.vector.tensor_copy(out=ones_col[:], in_=st1[:])
        nc.gpsimd.memset(ones_vcol[:], 1.0)
        st8 = cst.tile([8, 4, 128], F32, tag="st8")
        nc.gpsimd.memset(st8[:], 0.0)
        nc.gpsimd.affine_select(
            out=st8[:, :, 0:64], in_=st8[:, :, 0:64], compare_op=ALU.not_equal, fill=1.0,
            base=0, pattern=[[-2, 4], [0, 64]], channel_multiplier=1,
        )
        nc.gpsimd.affine_select(
            out=st8[:, :, 64:128], in_=st8[:, :, 64:128], compare_op=ALU.not_equal, fill=1.0,
            base=-1, pattern=[[-2, 4], [0, 64]], channel_multiplier=1,
        )
        nc.vector.tensor_copy(out=e8[:], in_=st8[:])

    # ---- persistent per-core buffers ----
    qfT = singles.tile([128, 4, s_total], DT_MM, name="qfT", tag="qfT")  # [inner, s]
    kvsb = singles.tile([128, 4, 128], DT_MM, name="kvsb", tag="kvsb")  # block-diag per chunk
    # block-diag Ksum rhs: [128, chunk, 8]; chunk c: rows 0-63 -> col 2c, rows 64-127 -> col 2c+1
    ksum_bd = singles.tile([128, 4, 8], DT_MM, name="ksum_bd", tag="ksum_bd")
    with tc.tile_pool(name="kbz_stage", bufs=2) as kbz:
        stz = kbz.tile([128, 4, 8], F32, tag="stz")
        nc.gpsimd.memset(stz[:], 0.0)
        nc.vector.tensor_copy(out=ksum_bd[:], in_=stz[:])
        stz2 = kbz.tile([128, 4, 128], F32, tag="stz2")
        nc.gpsimd.memset(stz2[:], 0.0)
        nc.vector.tensor_copy(out=kvsb[:], in_=stz2[:])

    def _passes():
        # =================== PASS A ===================
        with ExitStack() as actx:
            x_pool = actx.enter_context(tc.tile_pool(name="x_pool", bufs=4))
            xT_pool = actx.enter_context(tc.tile_pool(name="xT_pool", bufs=2))
            er_pool = actx.enter_context(tc.tile_pool(name="er_pool", bufs=6))
            kf_pool = actx.enter_context(tc.tile_pool(name="kf_pool", bufs=3))
            v_pool = actx.enter_context(tc.tile_pool(name="v_pool", bufs=3))
            ps_a = actx.enter_context(tc.tile_pool(name="ps_a", bufs=4, space="PSUM"))
            ps_acc = actx.enter_context(tc.tile_pool(name="ps_acc", bufs=1, space="PSUM"))

            # KV accumulators: quad g (m-cols 256g..256g+256 + ones col), chunk-in-quad cc
            kvq = [
                [
                    ps_acc.tile([128, 258], F32, name=f"kvq_{g}_{cc}", tag=f"kvq_{g}_{cc}")
                    for cc in range(2)
                ]
                for g in range(2)
            ]

            for ich in range(NCH):
                xT_t = xT_pool.tile([128, 4, 512], DT_MM, tag="xT")
                # ---- transpose x chunk (f32 PE transpose) ----
                for it in range(4):
                    ist = ich * 4 + it
                    xt = x_pool.tile([128, D], F32, tag="x")
                    nc.sync.dma_start(out=xt[:], in_=x_d[ist * 128 : (ist + 1) * 128, :])
                    xps = ps_a.tile([128, 4, 128], F32, tag="ps")
                    for c in range(4):
                        nc.tensor.transpose(xps[:, c, :], xt[:, c * 128 : (c + 1) * 128], ident[:])
                    nc.vector.tensor_copy(out=xT_t[:, :, it * 128 : (it + 1) * 128], in_=xps[:])
                # ---- qT = Wq^T xT, phi -> QfT ----
                for ci in range(4):
                    qps = ps_a.tile([128, 512], F32, tag="ps")
                    for cd in range(4):
                        nc.tensor.matmul(
                            qps[:],
                            lhsT=w_sb["Wq"][:, cd, ci * 128 : (ci + 1) * 128],
                            rhs=xT_t[:, cd, :],
                            start=(cd == 0),
                            stop=(cd == 3),
                        )
                    e_t = er_pool.tile([128, 512], F32, tag="er")
                    r_t = er_pool.tile([128, 512], F32, tag="er")
                    nc.scalar.activation(e_t[:], qps[:], AF.Exp, bias=bq_sb[:, ci : ci + 1], scale=1.0)
                    nc.scalar.activation(r_t[:], qps[:], AF.Relu, bias=bq_sb[:, ci : ci + 1], scale=1.0)
                    # phi = min(exp(x),1) + relu(x)
                    nc.vector.scalar_tensor_tensor(
                        out=qfT[:, ci, ich * 512 : (ich + 1) * 512],
                        in0=e_t[:],
                        scalar=1.0,
                        in1=r_t[:],
                        op0=ALU.min,
                        op1=ALU.add,
                    )
                # ---- k, v, KV accumulation per s-tile ----
                for it in range(4):
                    ist = ich * 4 + it
                    first, last = (ist == 0), (ist == NT - 1)
                    # k (natural layout) + bias via ones-row matmul
                    kps = ps_a.tile([128, 512], F32, tag="ps")
                    for cd in range(4):
                        nc.tensor.matmul(
                            kps[:],
                            lhsT=xT_t[:, cd, it * 128 : (it + 1) * 128],
                            rhs=w_sb["Wk"][:, cd, :],
                            start=(cd == 0),
                            stop=False,
                        )
                    nc.tensor.matmul(
                        kps[:], lhsT=ones_col[:], rhs=bk_row[:], start=False, stop=True
                    )
                    e_t = er_pool.tile([128, 512], F32, tag="er")
                    r_t = er_pool.tile([128, 512], F32, tag="er")
                    nc.scalar.activation(e_t[:], kps[:], AF.Exp)
                    nc.scalar.activation(r_t[:], kps[:], AF.Relu)
                    kf = kf_pool.tile([128, 512], DT_MM, tag="kf")
                    nc.vector.scalar_tensor_tensor(
                        out=kf[:], in0=e_t[:], scalar=1.0, in1=r_t[:], op0=ALU.min, op1=ALU.add
                    )
                    # v (natural) with bias fused into the psum->sbuf copy
                    vps = ps_a.tile([128, 512], F32, tag="ps")
                    for cd in range(4):
                        nc.tensor.matmul(
                            vps[:],
                            lhsT=xT_t[:, cd, it * 128 : (it + 1) * 128],
                            rhs=w_sb["Wv"][:, cd, :],
                            start=(cd == 0),
                            stop=(cd == 3),
                        )
                    vq = v_pool.tile([128, 2, 258], DT_MM, tag="v")
                    nc.vector.tensor_add(
                        out=vq[:, :, 0:256],
                        in0=vps[:].rearrange("p (g n) -> p g n", g=2),
                        in1=bv_rep[:].rearrange("p (g n) -> p g n", g=2),
                    )
                    nc.vector.tensor_copy(out=vq[:, :, 256:258], in_=ones_vcol[:])
                    # KV quad accumulation
                    for g in range(2):
                        for cc in range(2):
                            nc.tensor.matmul(
                                kvq[g][cc][:],
                                lhsT=kf[:, (2 * g + cc) * 128 : (2 * g + cc + 1) * 128],
                                rhs=vq[:, g, :],
                                start=first,
                                stop=last,
                            )

            # ---- extract KV blocks and Ksum (still inside pass-A pool scope) ----
            for h in range(H):
                g, cc = h // 4, (h // 2) % 2
                rh, qc = (h % 2) * 64, (h % 4) * 64
                nc.vector.tensor_copy(
                    out=kvsb[rh : rh + 64, h // 2, rh : rh + 64],
                    in_=kvq[g][cc][rh : rh + 64, qc : qc + 64],
                )
            for c in range(4):
                g, cc = c // 2, c % 2
                for half in range(2):
                    nc.vector.tensor_copy(
                        out=ksum_bd[half * 64 : (half + 1) * 64, c, 2 * c + half : 2 * c + half + 1],
                        in_=kvq[g][cc][half * 64 : (half + 1) * 64, 256:257],
                    )

        # ======================= PASS B =======================
        with ExitStack() as bctx:
            dz_ps = bctx.enter_context(tc.tile_pool(name="dz_ps", bufs=2, space="PSUM"))
            zrep_ps = bctx.enter_context(tc.tile_pool(name="zrep_ps", bufs=1, space="PSUM"))
            ot_ps = bctx.enter_context(tc.tile_pool(name="ot_ps", bufs=1, space="PSUM"))
            ow_ps = bctx.enter_context(tc.tile_pool(name="ow_ps", bufs=2, space="PSUM"))
            znat_pool = bctx.enter_context(tc.tile_pool(name="znat_pool", bufs=4))
            ztsb_pool = bctx.enter_context(tc.tile_pool(name="ztsb_pool", bufs=2))
            otsb_pool = bctx.enter_context(tc.tile_pool(name="otsb_pool", bufs=3))
            out_pool = bctx.enter_context(tc.tile_pool(name="out_pool", bufs=3))

            for isl in range(NSL):
                ztsb = ztsb_pool.tile([8, 2, 128], DT_MM, tag="ztsb")
                for half in range(2):
                    ist = isl * 2 + half
                    den = dz_ps.tile([128, 8], F32, tag="dz")
                    for c in range(4):
                        nc.tensor.matmul(
                            den[:],
                            lhsT=qfT[:, c, ist * 128 : (ist + 1) * 128],
                            rhs=ksum_bd[:, c, :],
                            start=(c == 0),
                            stop=(c == 3),
                        )
                    dtmp = znat_pool.tile([128, 8], F32, tag="znat")
                    nc.vector.tensor_scalar_add(out=dtmp[:], in0=den[:], scalar1=EPS)
                    znat = znat_pool.tile([128, 8], F32, tag="znat")
                    nc.vector.reciprocal(out=znat[:], in_=dtmp[:])
                    ztp = dz_ps.tile([8, 128], F32, tag="dz")
                    nc.tensor.transpose(ztp[:], znat[:], ident[:])
                    nc.vector.tensor_copy(out=ztsb[:, half, :], in_=ztp[:])
                # Zrep: replicate Z rows across head d-partitions (E8 selector matmul)
                zrep = zrep_ps.tile([128, 4, 256], F32, tag="zrep")
                for c in range(4):
                    nc.tensor.matmul(
                        zrep[:, c, :], lhsT=e8[:, c, :], rhs=ztsb[:], start=True, stop=True
                    )
                zrep_sb = otsb_pool.tile([128, 4, 256], F32, tag="zrep_sb")
                nc.vector.tensor_copy(out=zrep_sb[:], in_=zrep[:])
                # O^T per head pair (block-diagonal KV) then scale by Z
                otps = ot_ps.tile([128, 4, 256], F32, tag="ot")
                for c in range(4):
                    nc.tensor.matmul(
                        otps[:, c, :],
                        lhsT=kvsb[:, c, :],
                        rhs=qfT[:, c, isl * 256 : (isl + 1) * 256],
                        start=True,
                        stop=True,
                    )
                otsb = otsb_pool.tile([128, 4, 256], DT_MM, tag="otsb")
                for c in range(4):
                    nc.vector.tensor_mul(out=otsb[:, c, :], in0=otps[:, c, :], in1=zrep_sb[:, c, :])
                # out = OT^T Wo + bo
                for half in range(2):
                    ist = isl * 2 + half
                    owps = ow_ps.tile([128, 512], F32, tag="ow")
                    for c in range(4):
                        nc.tensor.matmul(
                            owps[:],
                            lhsT=otsb[:, c, half * 128 : (half + 1) * 128],
                            rhs=w_sb["Wo"][:, c, :],
                            start=(c == 0),
                            stop=(c == 3),
                        )
                    outt = out_pool.tile([128, 512], F32, tag="out")
                    nc.vector.tensor_add(out=outt[:], in0=owps[:], in1=bo_rep[:])
                    nc.sync.dma_start(out=out_d[ist * 128 : (ist + 1) * 128, :], in_=outt[:])


    if reps == 1:
        _passes()
    else:
        with tc.For_i(0, reps, 1):
            _passes()

def _legalize_waits(nc: "bass.Bass", max_waits: int = 1) -> int:
    """This toolchain's walrus allows at most ONE sync wait per instruction.

    Tile's scheduler attaches several; hoist the extras into standalone
    event-semaphore (pure wait) instructions on the same engine, placed
    immediately before the original — identical blocking semantics since
    waits execute in stream order on the issuing sequencer.
    """
    n_split = 0
    for func in nc.m.functions:
        for block in func.blocks:
            new_insts = []
            for inst in block.instructions:
                si = getattr(inst, "sync_info", None)
                waits = list(si.on_wait) if (si and si.on_wait) else []
                if len(waits) > max_waits:
                    extra, keep = waits[:-max_waits], waits[-max_waits:]
                    for j, w in enumerate(extra):
                        ev = mybir.InstEventSemaphore(
                            name=f"{inst.name}_lw{j}",
                            engine=inst.engine,
                            ins=[],
                            outs=[],
                            sync_info=mybir.SyncInfo(on_wait=[w], on_update=[]),
                        )
                        new_insts.append(ev)
                        n_split += 1
                    si.on_wait = keep
                new_insts.append(inst)
            block.instructions[:] = new_insts
    return n_split




def build_program(s_total: int = S, reps: int = 1) -> "bass.Bass":
    nc = bass.Bass("TRN2", target_bir_lowering=False, debug=False, num_devices=B)
    io = {
        "x": nc.dram_tensor("x", [s_total, D], F32, kind="ExternalInput").ap(),
        "Wq": nc.dram_tensor("Wq", [D, INNER], F32, kind="ExternalInput").ap(),
        "bq": nc.dram_tensor("bq", [INNER], F32, kind="ExternalInput").ap(),
        "Wk": nc.dram_tensor("Wk", [D, INNER], F32, kind="ExternalInput").ap(),
        "bk": nc.dram_tensor("bk", [INNER], F32, kind="ExternalInput").ap(),
        "Wv": nc.dram_tensor("Wv", [D, INNER], F32, kind="ExternalInput").ap(),
        "bv": nc.dram_tensor("bv", [INNER], F32, kind="ExternalInput").ap(),
        "Wo": nc.dram_tensor("Wo", [INNER, D], F32, kind="ExternalInput").ap(),
        "bo": nc.dram_tensor("bo", [D], F32, kind="ExternalInput").ap(),
        "out": nc.dram_tensor("out", [s_total, D], F32, kind="ExternalOutput").ap(),
    }
    with tile.TileContext(nc) as tc:
        with ExitStack() as ctx:
            _linattn_body(ctx, tc, io, s_total, reps=reps)
    return nc


_PROGRAM_CACHE: dict = {}


def _get_program(s_total: int = S) -> "bass.Bass":
    if s_total not in _PROGRAM_CACHE:
        nc = build_program(s_total)
        _legalize_waits(nc)
        _PROGRAM_CACHE[s_total] = nc
    return _PROGRAM_CACHE[s_total]


def _in_maps(inputs: dict) -> list:
    maps = []
    for b in range(B):
        m = {"x": np.ascontiguousarray(inputs["x"][b], dtype=np.float32)}
        for name in ("Wq", "bq", "Wk", "bk", "Wv", "bv", "Wo", "bo"):
            m[name] = np.ascontiguousarray(inputs[name], dtype=np.float32)
        maps.append(m)
    return maps


def run_hw(inputs: dict, trace: bool = False, **kwargs):
    """Run on the 8 NeuronCores. Returns (out [B,S,D], BassKernelResults)."""
    nc = _get_program(S)
    res = run_bass_kernel_spmd(nc, _in_maps(inputs), list(range(B)), trace=trace, **kwargs)
    out = np.stack([res.results[b]["out"] for b in range(B)], axis=0)
    return out, res


def kernel(**inputs) -> np.ndarray:
    out, _ = run_hw(inputs, trace=False)
    return out


def bench_hw(inputs: dict, iters: int = 20, nc_override=None):
    """Time repeated NEFF executions with device-resident inputs.

    Returns (per_iter_ns, out[B,S,D] from the first run). Uses the same
    shard_map lowering as run_bass_via_pjrt, without donation so input
    buffers can be reused across timed calls.
    """
    import time as _time

    import jax
    from jax.sharding import Mesh, NamedSharding, PartitionSpec
    from jax.experimental.shard_map import shard_map

    from concourse import bass2jax
    from concourse.bass2jax import _bass_exec_p, install_neuronx_cc_hook

    install_neuronx_cc_hook()
    nc = nc_override if nc_override is not None else _get_program(S)
    in_maps = _in_maps(inputs)

    partition_name = nc.partition_id_tensor.name if nc.partition_id_tensor else None
    in_names, out_names, out_avals = [], [], []
    for alloc in nc.m.functions[0].allocations:
        if not isinstance(alloc, mybir.MemoryLocationSet):
            continue
        name = alloc.memorylocations[0].name
        if alloc.kind == "ExternalInput":
            if name != partition_name:
                in_names.append(name)
        elif alloc.kind == "ExternalOutput":
            out_names.append(name)
            out_avals.append(
                jax.core.ShapedArray(tuple(alloc.tensor_shape), mybir.dt.np(alloc.dtype))
            )
    n_params = len(in_names)
    all_in_names = in_names + out_names
    if partition_name is not None:
        all_in_names = all_in_names + [partition_name]

    def _body(*args):
        operands = list(args)
        if partition_name is not None:
            operands.append(bass2jax.partition_id_tensor())
        outs = _bass_exec_p.bind(
            *operands,
            out_avals=tuple(out_avals),
            in_names=tuple(all_in_names),
            out_names=tuple(out_names),
            lowering_input_output_aliases=(),
            sim_require_finite=True,
            sim_require_nnan=True,
            nc=nc,
        )
        return tuple(outs)

    devices = jax.devices()[:B]
    mesh = Mesh(np.asarray(devices), ("core",))
    n_outs = len(out_names)
    in_specs = (PartitionSpec("core"),) * (n_params + n_outs)
    out_specs = (PartitionSpec("core"),) * n_outs
    fn = jax.jit(
        shard_map(_body, mesh=mesh, in_specs=in_specs, out_specs=out_specs, check_rep=False)
    )

    sh = NamedSharding(mesh, PartitionSpec("core"))
    concat_in = [
        jax.device_put(
            np.concatenate([np.asarray(in_maps[c][nm])[None] for c in range(B)], axis=0).reshape(
                B * np.asarray(in_maps[0][nm]).shape[0], *np.asarray(in_maps[0][nm]).shape[1:]
            ),
            sh,
        )
        for nm in in_names
    ]
    concat_zeros = [
        jax.device_put(np.zeros((B * a.shape[0], *a.shape[1:]), a.dtype), sh) for a in out_avals
    ]

    out = fn(*concat_in, *concat_zeros)
    jax.block_until_ready(out)
    first = np.asarray(out[0]).reshape(B, *out_avals[0].shape)

    def timed(f, n):
        t0 = _time.perf_counter()
        for _ in range(n):
            r = f(*concat_in, *concat_zeros)
        jax.block_until_ready(r)
        return (_time.perf_counter() - t0) / n

    timed(fn, 3)
    t = min(timed(fn, max(5, iters // 2)) for _ in range(4))
    return int(t * 1e9), first


def build_copy_program(s_total: int = S) -> "bass.Bass":
    """Same I/O signature as the real program, near-zero work: out = x."""
    nc = bass.Bass("TRN2", target_bir_lowering=False, debug=False, num_devices=B)
    io = {}
    io["x"] = nc.dram_tensor("x", [s_total, D], F32, kind="ExternalInput").ap()
    for nm, shp in (("Wq", [D, INNER]), ("bq", [INNER]), ("Wk", [D, INNER]), ("bk", [INNER]),
                    ("Wv", [D, INNER]), ("bv", [INNER]), ("Wo", [INNER, D]), ("bo", [D])):
        io[nm] = nc.dram_tensor(nm, shp, F32, kind="ExternalInput").ap()
    out_d = nc.dram_tensor("out", [s_total, D], F32, kind="ExternalOutput").ap()
    from contextlib import ExitStack as _ES
    with tile.TileContext(nc) as tc:
        with _ES() as ctx:
            pool = ctx.enter_context(tc.tile_pool(name="cp", bufs=4))
            for i in range(s_total // 128):
                t = pool.tile([128, D], F32, tag="cp")
                sl = slice(i * 128, (i + 1) * 128)
                nc.sync.dma_start(out=t[:], in_=io["x"][sl])
                nc.sync.dma_start(out=out_d[sl], in_=t[:])
    _legalize_waits(nc)
    return nc



# revision 44
# speedup vs baseline: 35.1354x; 35.1354x over previous
"""Trainium2 Bass kernel for LinearAttention (B=8, S=4096, D=512, H=8, DH=64).

Sharding: data-parallel over batch — core b processes batch element b end-to-end.

Per-core pipeline (all matmul operands bf16 — full 1 col/cycle PE rate at any
N, half-cost weight loads; f32 PSUM accumulation everywhere):
  pass A (per 512-wide s-chunk, software-pipelined: chunk i+1's transposes are
  emitted between chunk i's q-phase and k/v-phase so the PE never idles at
  chunk boundaries):
    x tile (f32, sync DMA queue) -> DVE cast bf16 -> PE transpose -> evac -> xT
    qT = Wq^T x^T (psum), phi -> QfT [inner, s]  (bias via ACT per-partition bias)
    k  = x Wk + bias (K=1 ones-row matmul), phi -> Kf [s, inner]
    v  = x Wv (plain cast evac; bv folded in later as the rank-1 term
        ksum (x) bv at KV extraction)
    KV' accumulated over all of S per (quad g, chunk cc) psum tile:
        KVq[g,cc] += Kf[:,chunk]^T @ v'[:,quad]   (ones col produces Ksum)
    phi = min(exp(x),1) + relu(x) = elu(x)+1, via 2 ACT ops + 1 DVE STT
  pass B (per 128-wide s-tile; every psum tile is one bank so all pools
  double-buffer within the 8-bank budget):
    den = QfT^T ksum_bd [s, 8] (block-diag rhs), Z = 1/den (eps dropped:
        den ~1e5 >> 1e-6), Z^T via PE transpose, Zrep via E8 selector matmul,
        evacuated on ACT (idle in pass B; DVE may read only one PSUM operand)
    O^T = KV^T @ QfT per head (block-diag kvsb), OT = O^T * Zrep (DVE)
    out = OT^T Wo, bo added in the DVE psum evac -> DMA (sync queue)
Weights load on the scalar HWDGE queue; all setup staging lives in the
persistent pool (scoped-pool SBUF would be recycled by pass-A pools, and the
WAR deps serialize kernel start ~20us behind the setup chain).
"""

import os
import sys

import numpy as np

for _p in ("/opt/trn_rl_repo",):
    if os.path.isdir(_p) and _p not in sys.path:
        sys.path.insert(0, _p)

from contextlib import ExitStack

import concourse.bass as bass
import concourse.mybir as mybir
import concourse.tile as tile
from concourse.bass_utils import run_bass_kernel_spmd
from concourse.masks import make_identity

B, S, D = 8, 4096, 512
H, DH = 8, 64
INNER = H * DH  # 512

F32 = mybir.dt.float32
F32R = mybir.dt.float32r
BF16 = mybir.dt.bfloat16
AF = mybir.ActivationFunctionType
ALU = mybir.AluOpType

MM_DTYPE = os.environ.get("LINATTN_MM_DTYPE", "bf16")
DT_MM = BF16 if MM_DTYPE == "bf16" else F32R


def _linattn_body(ctx: ExitStack, tc: "tile.TileContext", io: dict, s_total: int, reps: int = 1):
    nc = tc.nc
    NT = s_total // 128  # s-tiles
    NCH = s_total // 512  # pass-A chunks
    NSL = s_total // 256  # pass-B slices

    x_d = io["x"]
    out_d = io["out"]

    singles = ctx.enter_context(tc.tile_pool(name="singles", bufs=1))

    # ---- weights: [128, 4, 512] in DT_MM; loaded on the scalar HWDGE queue so
    # the sync queue is free for x tiles from the first instruction. Staging
    # lives in the persistent pool: a scoped pool's SBUF would be recycled by
    # the pass-A pools, and that WAR dependency stalls the first transposes
    # ~25us behind the weight-cast chain. ----
    w_sb = {}
    wraw = singles.tile([128, 2, 4, INNER], F32, name="wraw", tag="wraw")
    for wi, name in enumerate(("Wq", "Wk", "Wv", "Wo")):
        raw = wraw[:, wi % 2]
        w3 = io[name].rearrange("(c p) n -> c p n", p=128)
        t = singles.tile([128, 4, INNER], DT_MM, name=f"{name}_sb", tag=f"{name}_sb")
        for c in range(4):
            nc.scalar.dma_start(out=raw[:, c, :], in_=w3[c])
            nc.any.tensor_copy(out=t[:, c, :], in_=raw[:, c, :])
        w_sb[name] = t
    bk_raw = singles.tile([1, INNER], F32, name="bk_raw", tag="bk_raw")
    nc.gpsimd.dma_start(out=bk_raw[:], in_=io["bk"].rearrange("(a n) -> a n", a=1))
    bk_row = singles.tile([1, INNER], DT_MM, name="bk_row", tag="bk_row")
    nc.vector.tensor_copy(out=bk_row[:], in_=bk_raw[:])

    # ---- biases ----
    bq_sb = singles.tile([128, 4], F32, name="bq_sb", tag="bq_sb")
    nc.gpsimd.dma_start(out=bq_sb[:], in_=io["bq"].rearrange("(c p) -> p c", p=128))
    # bv/bo replicated across partitions via partition-step-0 DMA
    bv_rep = singles.tile([128, INNER], F32, name="bv_rep", tag="bv_rep")
    bv_ap = io["bv"]
    nc.gpsimd.dma_start(
        out=bv_rep[:],
        in_=bass.AP(tensor=bv_ap.tensor, offset=bv_ap.offset, ap=[[0, 128]] + list(bv_ap.ap)),
    )
    bo_rep = singles.tile([128, D], F32, name="bo_rep", tag="bo_rep")
    bo_ap = io["bo"]
    nc.gpsimd.dma_start(
        out=bo_rep[:],
        in_=bass.AP(tensor=bo_ap.tensor, offset=bo_ap.offset, ap=[[0, 128]] + list(bo_ap.ap)),
    )

    # ---- constants ----
    ident = singles.tile([128, 128], F32, name="ident", tag="ident")
    make_identity(nc, ident[:])
    ident_bf = singles.tile([128, 128], DT_MM, name="ident_bf", tag="ident_bf")
    nc.vector.tensor_copy(out=ident_bf[:], in_=ident[:])
    ones_col = singles.tile([1, 128], DT_MM, name="ones_col", tag="ones_col")
    ones_vcol = singles.tile([128, 2, 2], DT_MM, name="ones_vcol", tag="ones_vcol")
    e8 = singles.tile([8, 4, 128], DT_MM, name="e8", tag="e8")
    # staging lives in the persistent pool: scoped-pool SBUF would be recycled
    # by the pass-A pools, and those WAR deps serialize pass-A start behind the
    # whole setup chain (measured ~20us of dead time)
    st1 = singles.tile([1, 128], F32, name="st1", tag="st1")
    nc.gpsimd.memset(st1[:], 1.0)
    nc.vector.tensor_copy(out=ones_col[:], in_=st1[:])
    stv = singles.tile([128, 2, 2], F32, name="stv", tag="stv")
    nc.gpsimd.memset(stv[:], 1.0)
    nc.vector.tensor_copy(out=ones_vcol[:], in_=stv[:])
    st8 = singles.tile([8, 4, 128], F32, name="st8", tag="st8")
    nc.gpsimd.memset(st8[:], 0.0)
    nc.gpsimd.affine_select(
        out=st8[:, :, 0:64], in_=st8[:, :, 0:64], compare_op=ALU.not_equal, fill=1.0,
        base=0, pattern=[[-2, 4], [0, 64]], channel_multiplier=1,
    )
    nc.gpsimd.affine_select(
        out=st8[:, :, 64:128], in_=st8[:, :, 64:128], compare_op=ALU.not_equal, fill=1.0,
        base=-1, pattern=[[-2, 4], [0, 64]], channel_multiplier=1,
    )
    nc.vector.tensor_copy(out=e8[:], in_=st8[:])

    # ---- persistent per-core buffers ----
    qfT = singles.tile([128, 4, s_total], DT_MM, name="qfT", tag="qfT")  # [inner, s]
    kvsb = singles.tile([128, 4, 128], DT_MM, name="kvsb", tag="kvsb")  # block-diag per chunk
    # block-diag Ksum rhs: [128, chunk, 8]; chunk c: rows 0-63 -> col 2c, rows 64-127 -> col 2c+1
    ksum_bd = singles.tile([128, 4, 8], DT_MM, name="ksum_bd", tag="ksum_bd")
    stz = singles.tile([128, 4, 8], F32, name="stz", tag="stz")
    nc.gpsimd.memset(stz[:], 0.0)
    nc.vector.tensor_copy(out=ksum_bd[:], in_=stz[:])
    stz2 = singles.tile([128, 4, 128], F32, name="stz2", tag="stz2")
    nc.gpsimd.memset(stz2[:], 0.0)
    nc.vector.tensor_copy(out=kvsb[:], in_=stz2[:])

    def _passes():
        # =================== PASS A ===================
        with ExitStack() as actx:
            x_pool = actx.enter_context(tc.tile_pool(name="x_pool", bufs=6))
            xb_pool = actx.enter_context(tc.tile_pool(name="xb_pool", bufs=4))
            xT_pool = actx.enter_context(tc.tile_pool(name="xT_pool", bufs=2))
            er_pool = actx.enter_context(tc.tile_pool(name="er_pool", bufs=8))
            kf_pool = actx.enter_context(tc.tile_pool(name="kf_pool", bufs=4))
            v_pool = actx.enter_context(tc.tile_pool(name="v_pool", bufs=4))
            ps_a = actx.enter_context(tc.tile_pool(name="ps_a", bufs=4, space="PSUM"))
            ps_acc = actx.enter_context(tc.tile_pool(name="ps_acc", bufs=1, space="PSUM"))

            # KV accumulators: quad g (m-cols 256g..256g+256 + ones col), chunk-in-quad cc
            kvq = [
                [
                    ps_acc.tile([128, 258], F32, name=f"kvq_{g}_{cc}", tag=f"kvq_{g}_{cc}")
                    for cc in range(2)
                ]
                for g in range(2)
            ]

            def _transpose_tile(xT_t, ich, it):
                # f32->bf16 cast on DVE, then bf16 PE transpose (1 cyc/row).
                # (XBAR DMA transpose measured 1.2us per [128,128] tile — 6x
                # slower than the PE path; not worth the freed PE cycles.)
                ist = ich * 4 + it
                xt = x_pool.tile([128, D], F32, tag="x")
                nc.sync.dma_start(out=xt[:], in_=x_d[ist * 128 : (ist + 1) * 128, :])
                xb = xb_pool.tile([128, D], DT_MM, tag="xb")
                nc.vector.tensor_copy(out=xb[:], in_=xt[:])
                xps = ps_a.tile([128, 4, 128], DT_MM, tag="ps")
                for c in range(4):
                    nc.tensor.transpose(xps[:, c, :], xb[:, c * 128 : (c + 1) * 128], ident_bf[:])
                nc.vector.tensor_copy(out=xT_t[:, :, it * 128 : (it + 1) * 128], in_=xps[:])

            xT_t = xT_pool.tile([128, 4, 512], DT_MM, tag="xT")
            for it in range(4):
                _transpose_tile(xT_t, 0, it)
            for ich in range(NCH):
                xT_cur = xT_t
                if ich + 1 < NCH:
                    xT_t = xT_pool.tile([128, 4, 512], DT_MM, tag="xT")
                # per-tile interleave of q / next-chunk-transpose / k/v/KV:
                # same-kind psum tiles recur at full-iteration distance and the
                # phi ACTs alternate q/k in the FIFO, so no engine builds up a
                # block of same-kind dependencies (block-ordered emission
                # stalled the PE on the psum-WAR -> phi-ACT chain)
                for it in range(4):
                    ist = ich * 4 + it
                    # ---- qT = Wq^T xT, phi -> QfT (ci = it) ----
                    ci = it
                    qps = ps_a.tile([128, 512], F32, tag="ps")
                    for cd in range(4):
                        nc.tensor.matmul(
                            qps[:],
                            lhsT=w_sb["Wq"][:, cd, ci * 128 : (ci + 1) * 128],
                            rhs=xT_cur[:, cd, :],
                            start=(cd == 0),
                            stop=(cd == 3),
                        )
                    e_t = er_pool.tile([128, 512], DT_MM, tag="er")
                    r_t = er_pool.tile([128, 512], DT_MM, tag="er")
                    nc.scalar.activation(e_t[:], qps[:], AF.Exp, bias=bq_sb[:, ci : ci + 1], scale=1.0)
                    nc.scalar.activation(r_t[:], qps[:], AF.Relu, bias=bq_sb[:, ci : ci + 1], scale=1.0)
                    # phi = min(exp(x),1) + relu(x)
                    nc.vector.scalar_tensor_tensor(
                        out=qfT[:, ci, ich * 512 : (ich + 1) * 512],
                        in0=e_t[:],
                        scalar=1.0,
                        in1=r_t[:],
                        op0=ALU.min,
                        op1=ALU.add,
                    )
                    # ---- next chunk's transpose for this tile position ----
                    if ich + 1 < NCH:
                        _transpose_tile(xT_t, ich + 1, it)
                    first, last = (ist == 0), (ist == NT - 1)
                    # k (natural layout) + bias via ones-row matmul
                    kps = ps_a.tile([128, 512], F32, tag="ps")
                    for cd in range(4):
                        nc.tensor.matmul(
                            kps[:],
                            lhsT=xT_cur[:, cd, it * 128 : (it + 1) * 128],
                            rhs=w_sb["Wk"][:, cd, :],
                            start=(cd == 0),
                            stop=False,
                        )
                    nc.tensor.matmul(
                        kps[:], lhsT=ones_col[:], rhs=bk_row[:], start=False, stop=True
                    )
                    e_t = er_pool.tile([128, 512], DT_MM, tag="er")
                    r_t = er_pool.tile([128, 512], DT_MM, tag="er")
                    nc.scalar.activation(e_t[:], kps[:], AF.Exp)
                    nc.scalar.activation(r_t[:], kps[:], AF.Relu)
                    kf = kf_pool.tile([128, 512], DT_MM, tag="kf")
                    nc.vector.scalar_tensor_tensor(
                        out=kf[:], in0=e_t[:], scalar=1.0, in1=r_t[:], op0=ALU.min, op1=ALU.add
                    )
                    # v (natural) with bias fused into the psum->sbuf copy
                    vps = ps_a.tile([128, 512], F32, tag="ps")
                    for cd in range(4):
                        nc.tensor.matmul(
                            vps[:],
                            lhsT=xT_cur[:, cd, it * 128 : (it + 1) * 128],
                            rhs=w_sb["Wv"][:, cd, :],
                            start=(cd == 0),
                            stop=(cd == 3),
                        )
                    vq = v_pool.tile([128, 2, 258], DT_MM, tag="v")
                    # bv is NOT added here: KV of (v + bv) = KV(v) + ksum (x) bv,
                    # a rank-1 term applied once at extraction (saves a DVE
                    # tensor-tensor per tile; the evac is a plain cast)
                    nc.vector.tensor_copy(
                        out=vq[:, :, 0:256],
                        in_=vps[:].rearrange("p (g n) -> p g n", g=2),
                    )
                    nc.vector.tensor_copy(out=vq[:, :, 256:258], in_=ones_vcol[:])
                    # KV quad accumulation
                    for g in range(2):
                        for cc in range(2):
                            nc.tensor.matmul(
                                kvq[g][cc][:],
                                lhsT=kf[:, (2 * g + cc) * 128 : (2 * g + cc + 1) * 128],
                                rhs=vq[:, g, :],
                                start=first,
                                stop=last,
                            )

            # ---- extract Ksum then KV blocks (still inside pass-A pool scope);
            # the KV copy applies the deferred rank-1 v-bias: ksum (x) bv ----
            for c in range(4):
                g, cc = c // 2, c % 2
                for half in range(2):
                    nc.vector.tensor_copy(
                        out=ksum_bd[half * 64 : (half + 1) * 64, c, 2 * c + half : 2 * c + half + 1],
                        in_=kvq[g][cc][half * 64 : (half + 1) * 64, 256:257],
                    )
            for h in range(H):
                g, cc = h // 4, (h // 2) % 2
                rh, qc = (h % 2) * 64, (h % 4) * 64
                nc.vector.scalar_tensor_tensor(
                    out=kvsb[rh : rh + 64, h // 2, rh : rh + 64],
                    in0=bv_rep[rh : rh + 64, h * 64 : (h + 1) * 64],
                    scalar=ksum_bd[rh : rh + 64, h // 2, 2 * (h // 2) + (h % 2) : 2 * (h // 2) + (h % 2) + 1],
                    in1=kvq[g][cc][rh : rh + 64, qc : qc + 64],
                    op0=ALU.mult,
                    op1=ALU.add,
                )

        # ======================= PASS B =======================
        # 128-wide slices: every psum tile is exactly one bank, so all four
        # pools double-buffer within the 8-bank budget (256-wide slices forced
        # single-buffered zrep/ot tiles, serializing each slice ~2.4us on the
        # evac/mul reads)
        with ExitStack() as bctx:
            den_ps = bctx.enter_context(tc.tile_pool(name="den_ps", bufs=1, space="PSUM"))
            ztp_ps = bctx.enter_context(tc.tile_pool(name="ztp_ps", bufs=1, space="PSUM"))
            zrep_ps = bctx.enter_context(tc.tile_pool(name="zrep_ps", bufs=2, space="PSUM"))
            ot_ps = bctx.enter_context(tc.tile_pool(name="ot_ps", bufs=2, space="PSUM"))
            ow_ps = bctx.enter_context(tc.tile_pool(name="ow_ps", bufs=2, space="PSUM"))
            znat_pool = bctx.enter_context(tc.tile_pool(name="znat_pool", bufs=4))
            ztsb_pool = bctx.enter_context(tc.tile_pool(name="ztsb_pool", bufs=3))
            otsb_pool = bctx.enter_context(tc.tile_pool(name="otsb_pool", bufs=4))
            out_pool = bctx.enter_context(tc.tile_pool(name="out_pool", bufs=4))

            for ist in range(NT):
                sl = slice(ist * 128, (ist + 1) * 128)
                # den [s, 8]; Z = 1/den natural (cheap recip shape); eps dropped
                # (den ~ 1e5 >> 1e-6)
                den = den_ps.tile([128, 8], F32, tag="den")
                for c in range(4):
                    nc.tensor.matmul(
                        den[:],
                        lhsT=qfT[:, c, sl],
                        rhs=ksum_bd[:, c, :],
                        start=(c == 0),
                        stop=(c == 3),
                    )
                znat = znat_pool.tile([128, 8], F32, tag="znat")
                nc.vector.reciprocal(out=znat[:], in_=den[:])
                ztp = ztp_ps.tile([8, 128], F32, tag="ztp")
                nc.tensor.transpose(ztp[:], znat[:], ident[:])
                ztsb = ztsb_pool.tile([8, 128], DT_MM, tag="ztsb")
                nc.vector.tensor_copy(out=ztsb[:], in_=ztp[:])
                # Zrep: replicate Z rows across head d-partitions (E8 selector)
                zrep = zrep_ps.tile([128, 4, 128], F32, tag="zrep")
                for c in range(4):
                    nc.tensor.matmul(
                        zrep[:, c, :], lhsT=e8[:, c, :], rhs=ztsb[:], start=True, stop=True
                    )
                # DVE can read only ONE operand from PSUM: evacuate Zrep on the
                # scalar engine (idle in pass B)
                zrep_sb = otsb_pool.tile([128, 4, 128], F32, tag="zrep_sb")
                nc.scalar.activation(zrep_sb[:], zrep[:], AF.Copy)
                # O^T per head pair (block-diagonal KV) then scale by Z
                otps = ot_ps.tile([128, 4, 128], F32, tag="ot")
                for c in range(4):
                    nc.tensor.matmul(
                        otps[:, c, :],
                        lhsT=kvsb[:, c, :],
                        rhs=qfT[:, c, sl],
                        start=True,
                        stop=True,
                    )
                otsb = otsb_pool.tile([128, 4, 128], DT_MM, tag="otsb")
                nc.vector.tensor_mul(out=otsb[:], in0=otps[:], in1=zrep_sb[:])
                # out = OT^T Wo; bo added in the psum evac; store on sync queue
                owps = ow_ps.tile([128, 512], F32, tag="ow")
                for c in range(4):
                    nc.tensor.matmul(
                        owps[:],
                        lhsT=otsb[:, c, :],
                        rhs=w_sb["Wo"][:, c, :],
                        start=(c == 0),
                        stop=(c == 3),
                    )
                outt = out_pool.tile([128, 512], F32, tag="out")
                nc.vector.tensor_add(out=outt[:], in0=owps[:], in1=bo_rep[:])
                nc.sync.dma_start(out=out_d[sl, :], in_=outt[:])

    if reps == 1:
        _passes()
    elif os.environ.get("LINATTN_UNROLL") == "1":
        for _ in range(reps):
            _passes()
    else:
        with tc.For_i(0, reps, 1):
            _passes()


def _legalize_waits(nc: "bass.Bass", max_waits: int = 1) -> int:
    """This toolchain's walrus allows at most ONE sync wait per instruction.

    Tile's scheduler attaches several; hoist the extras into standalone
    event-semaphore (pure wait) instructions on the same engine, placed
    immediately before the original — identical blocking semantics since
    waits execute in stream order on the issuing sequencer.
    """
    n_split = 0
    for func in nc.m.functions:
        for block in func.blocks:
            new_insts = []
            for inst in block.instructions:
                si = getattr(inst, "sync_info", None)
                waits = list(si.on_wait) if (si and si.on_wait) else []
                if len(waits) > max_waits:
                    extra, keep = waits[:-max_waits], waits[-max_waits:]
                    for j, w in enumerate(extra):
                        ev = mybir.InstEventSemaphore(
                            name=f"{inst.name}_lw{j}",
                            engine=inst.engine,
                            ins=[],
                            outs=[],
                            sync_info=mybir.SyncInfo(on_wait=[w], on_update=[]),
                        )
                        new_insts.append(ev)
                        n_split += 1
                    si.on_wait = keep
                new_insts.append(inst)
            block.instructions[:] = new_insts
    return n_split


def build_program(s_total: int = S, reps: int = 1) -> "bass.Bass":
    nc = bass.Bass("TRN2", target_bir_lowering=False, debug=False, num_devices=B)
    io = {
        "x": nc.dram_tensor("x", [s_total, D], F32, kind="ExternalInput").ap(),
        "Wq": nc.dram_tensor("Wq", [D, INNER], F32, kind="ExternalInput").ap(),
        "bq": nc.dram_tensor("bq", [INNER], F32, kind="ExternalInput").ap(),
        "Wk": nc.dram_tensor("Wk", [D, INNER], F32, kind="ExternalInput").ap(),
        "bk": nc.dram_tensor("bk", [INNER], F32, kind="ExternalInput").ap(),
        "Wv": nc.dram_tensor("Wv", [D, INNER], F32, kind="ExternalInput").ap(),
        "bv": nc.dram_tensor("bv", [INNER], F32, kind="ExternalInput").ap(),
        "Wo": nc.dram_tensor("Wo", [INNER, D], F32, kind="ExternalInput").ap(),
        "bo": nc.dram_tensor("bo", [D], F32, kind="ExternalInput").ap(),
        "out": nc.dram_tensor("out", [s_total, D], F32, kind="ExternalOutput").ap(),
    }
    with tile.TileContext(nc) as tc:
        with ExitStack() as ctx:
            _linattn_body(ctx, tc, io, s_total, reps=reps)
    return nc


_PROGRAM_CACHE: dict = {}


def _get_program(s_total: int = S) -> "bass.Bass":
    if s_total not in _PROGRAM_CACHE:
        nc = build_program(s_total)
        _legalize_waits(nc)
        _PROGRAM_CACHE[s_total] = nc
    return _PROGRAM_CACHE[s_total]


def _in_maps(inputs: dict) -> list:
    maps = []
    for b in range(B):
        m = {"x": np.ascontiguousarray(inputs["x"][b], dtype=np.float32)}
        for name in ("Wq", "bq", "Wk", "bk", "Wv", "bv", "Wo", "bo"):
            m[name] = np.ascontiguousarray(inputs[name], dtype=np.float32)
        maps.append(m)
    return maps


def run_hw(inputs: dict, trace: bool = False, **kwargs):
    """Run on the 8 NeuronCores. Returns (out [B,S,D], BassKernelResults)."""
    nc = _get_program(S)
    res = run_bass_kernel_spmd(nc, _in_maps(inputs), list(range(B)), trace=trace, **kwargs)
    out = np.stack([res.results[b]["out"] for b in range(B)], axis=0)
    return out, res


def kernel(**inputs) -> np.ndarray:
    out, _ = run_hw(inputs, trace=False)
    return out


def bench_hw(inputs: dict, iters: int = 20, nc_override=None):
    """Time repeated NEFF executions with device-resident inputs.

    Returns (per_iter_ns, out[B,S,D] from the first run). Uses the same
    shard_map lowering as run_bass_via_pjrt, without donation so input
    buffers can be reused across timed calls.
    """
    import time as _time

    import jax
    from jax.sharding import Mesh, NamedSharding, PartitionSpec
    from jax.experimental.shard_map import shard_map

    from concourse import bass2jax
    from concourse.bass2jax import _bass_exec_p, install_neuronx_cc_hook

    install_neuronx_cc_hook()
    nc = nc_override if nc_override is not None else _get_program(S)
    in_maps = _in_maps(inputs)

    partition_name = nc.partition_id_tensor.name if nc.partition_id_tensor else None
    in_names, out_names, out_avals = [], [], []
    for alloc in nc.m.functions[0].allocations:
        if not isinstance(alloc, mybir.MemoryLocationSet):
            continue
        name = alloc.memorylocations[0].name
        if alloc.kind == "ExternalInput":
            if name != partition_name:
                in_names.append(name)
        elif alloc.kind == "ExternalOutput":
            out_names.append(name)
            out_avals.append(
                jax.core.ShapedArray(tuple(alloc.tensor_shape), mybir.dt.np(alloc.dtype))
            )
    n_params = len(in_names)
    all_in_names = in_names + out_names
    if partition_name is not None:
        all_in_names = all_in_names + [partition_name]

    def _body(*args):
        operands = list(args)
        if partition_name is not None:
            operands.append(bass2jax.partition_id_tensor())
        outs = _bass_exec_p.bind(
            *operands,
            out_avals=tuple(out_avals),
            in_names=tuple(all_in_names),
            out_names=tuple(out_names),
            lowering_input_output_aliases=(),
            sim_require_finite=True,
            sim_require_nnan=True,
            nc=nc,
        )
        return tuple(outs)

    devices = jax.devices()[:B]
    mesh = Mesh(np.asarray(devices), ("core",))
    n_outs = len(out_names)
    in_specs = (PartitionSpec("core"),) * (n_params + n_outs)
    out_specs = (PartitionSpec("core"),) * n_outs
    fn = jax.jit(
        shard_map(_body, mesh=mesh, in_specs=in_specs, out_specs=out_specs, check_rep=False)
    )

    sh = NamedSharding(mesh, PartitionSpec("core"))
    concat_in = [
        jax.device_put(
            np.concatenate([np.asarray(in_maps[c][nm])[None] for c in range(B)], axis=0).reshape(
                B * np.asarray(in_maps[0][nm]).shape[0], *np.asarray(in_maps[0][nm]).shape[1:]
            ),
            sh,
        )
        for nm in in_names
    ]
    concat_zeros = [
        jax.device_put(np.zeros((B * a.shape[0], *a.shape[1:]), a.dtype), sh) for a in out_avals
    ]

    out = fn(*concat_in, *concat_zeros)
    jax.block_until_ready(out)
    first = np.asarray(out[0]).reshape(B, *out_avals[0].shape)

    def timed(f, n):
        t0 = _time.perf_counter()
        for _ in range(n):
            r = f(*concat_in, *concat_zeros)
        jax.block_until_ready(r)
        return (_time.perf_counter() - t0) / n

    timed(fn, 3)
    t = min(timed(fn, max(5, iters // 2)) for _ in range(4))
    return int(t * 1e9), first


def build_copy_program(s_total: int = S) -> "bass.Bass":
    """Same I/O signature as the real program, near-zero work: out = x."""
    nc = bass.Bass("TRN2", target_bir_lowering=False, debug=False, num_devices=B)
    io = {}
    io["x"] = nc.dram_tensor("x", [s_total, D], F32, kind="ExternalInput").ap()
    for nm, shp in (("Wq", [D, INNER]), ("bq", [INNER]), ("Wk", [D, INNER]), ("bk", [INNER]),
                    ("Wv", [D, INNER]), ("bv", [INNER]), ("Wo", [INNER, D]), ("bo", [D])):
        io[nm] = nc.dram_tensor(nm, shp, F32, kind="ExternalInput").ap()
    out_d = nc.dram_tensor("out", [s_total, D], F32, kind="ExternalOutput").ap()
    from contextlib import ExitStack as _ES
    with tile.TileContext(nc) as tc:
        with _ES() as ctx:
            pool = ctx.enter_context(tc.tile_pool(name="cp", bufs=4))
            for i in range(s_total // 128):
                t = pool.tile([128, D], F32, tag="cp")
                sl = slice(i * 128, (i + 1) * 128)
                nc.sync.dma_start(out=t[:], in_=io["x"][sl])
                nc.sync.dma_start(out=out_d[sl], in_=t[:])
    _legalize_waits(nc)
    return nc
